# revision 15
# baseline (speedup 1.0000x reference)
"""Trainium2 Bass kernel for nn_DualModalHyperGraph (dual-modal hypergraph conv).

Self-contained: builds one SPMD Bass/Tile program for 8 NeuronCores, shards
inputs on the host, runs via run_bass_kernel_spmd, reassembles the output.

Math (equivalent to the reference):
  sim_m = col-normalized(mean_B feat_m) gram matrix     (per modality m)
  M_mk[n, r] = 1 iff r in top-(k+1) of row n            (two k per modality)
  Sigma_m = sum_k M_mk^T M_mk / (k+1)^2                 ([2048, 2048])
  S = blkdiag(Sigma_1, Sigma_2) + 1/4 [[I,I],[I,I]]
  A = D^-1/2 S D^-1/2,  D = diag(rowsum of H)
  x1 = relu(A (x @ W1^T)); x2 = relu(A (x1 @ W2^T))

Distribution (core c = 4*m + s: modality m, 512-row slab s):
  - Each core computes sim rows + top-k masks for its own 512 rows only.
  - Partial Sigma_m (contraction over own rows) is computed locally from
    scaled fp16 masks and summed via a per-modality ReduceScatter that also
    carries partial column-sum (degree) vectors; each core receives
    Sigma_m[own 512 rows, all 2048 modality cols] (slabT) plus its degrees.
  - Both hgconv layers are computed as partial contributions over own rows
    (y += S[own, :]^T u[own]) summed by 8-core ReduceScatters; the J = 1/4
    inter-modality block is applied via host-gated 0.25-identity matmuls.
  - SPMD divergence is data-driven only (host flag/diagonal tensors).
"""

import numpy as np

import concourse.bass as bass
import concourse.bacc as bacc
import concourse.mybir as mybir
import concourse.tile as tile
from concourse.bass_utils import run_bass_kernel_spmd
from concourse.masks import make_identity

P = 128
B = 4
N = 2048          # nodes per modality
C = 64
F = 128
NT = 16           # 128-row tiles per modality
GT = 32           # global row tiles
BF = B * F        # 512
MINVAL = -3.0e38
F16 = mybir.dt.float16
F32 = mybir.dt.float32
FP8 = mybir.dt.float8e4
AL = mybir.AluOpType
AF = mybir.ActivationFunctionType

SIGD = 4 * P * N          # per-dest Sigma elems in the RS-Sigma buffer
CSD = 2 * 512             # per-dest colsum elems
DD = SIGD + CSD

DEBUG = False
NO_COLLECTIVES = False   # timing experiment: skip RS (results become garbage)
_CACHED_NC = None
LAST_EXEC_TIME_NS = None
LAST_RESULTS = None


def build_nc():
    nc = bacc.Bacc("TRN2", target_bir_lowering=False, debug=False, num_devices=8)

    xTm = nc.dram_tensor("xTm", [C, B, N], F32, kind="ExternalInput")
    xTmy = nc.dram_tensor("xTmy", [C, B, 512], F32, kind="ExternalInput")
    w1t = nc.dram_tensor("w1t", [C, F], F32, kind="ExternalInput")
    w2t = nc.dram_tensor("w2t", [F, F], F16, kind="ExternalInput")
    slotmask = nc.dram_tensor("slotmask", [P, 24], F32, kind="ExternalInput")
    cconst = nc.dram_tensor("cconst", [P, 8], F32, kind="ExternalInput")
    jdiag = nc.dram_tensor("jdiag", [P, GT * P], F16, kind="ExternalInput")
    out_z = nc.dram_tensor("out_z", [B, 512, F], F32, kind="ExternalOutput")
    if DEBUG:
        dbg_mk = nc.dram_tensor("dbg_mk", [P, 8 * N], FP8, kind="ExternalOutput")
        dbg_slabT = nc.dram_tensor("dbg_slabT", [P, 4 * N], F16, kind="ExternalOutput")
        dbg_cs = nc.dram_tensor("dbg_cs", [P, 8], F32, kind="ExternalOutput")
        dbg_d = nc.dram_tensor("dbg_d", [P, 8], F32, kind="ExternalOutput")
        dbg_u1d = nc.dram_tensor("dbg_u1d", [P, 4 * BF], F16, kind="ExternalOutput")
        dbg_y1 = nc.dram_tensor("dbg_y1", [P, 4 * BF], F16, kind="ExternalOutput")
        dbg_rsin1 = nc.dram_tensor("dbg_rsin1", [GT, P * BF], F16, kind="ExternalOutput")

    csflat_d = nc.dram_tensor("csflat_d", [1, 2 * N], F16)
    rsinS = nc.dram_tensor("rsinS", [4, DD], F16)
    rsoutS = nc.dram_tensor("rsoutS", [1, DD], F16)
    rsin1 = nc.dram_tensor("rsin1", [8, 4, P, BF], F16)
    rsout1 = nc.dram_tensor("rsout1", [4, P, BF], F16)
    rsin2 = nc.dram_tensor("rsin2", [8, 4, P, BF], F16)
    rsout2 = nc.dram_tensor("rsout2", [4, P, BF], F16)

    ALLW = [list(range(8))]
    MODW = [[0, 1, 2, 3], [4, 5, 6, 7]]

    # dram views
    sig_in = rsinS[:, 0:SIGD].rearrange("d (t p n) -> d t p n", t=4, p=P)
    cs_in = rsinS[:, SIGD:DD]                       # [4, 1024]
    sig_out = rsoutS[:, 0:SIGD].rearrange("a (t p n) -> (a t) p n", t=4, p=P)
    cs_out = rsoutS[:, SIGD:DD]                     # [1, 1024]

    with tile.TileContext(nc) as tc:
      with tc.tile_pool(name="persist", bufs=1) as pp:
        cc = pp.tile([P, 8], F32)
        sm = pp.tile([P, 24], F32)
        jdg = pp.tile([P, GT, P], F16)
        w1s = pp.tile([C, F], F32)
        w2s = pp.tile([F, F], F16)
        nc.sync.dma_start(cc[:], cconst[:])
        nc.sync.dma_start(sm[:], slotmask[:])
        nc.sync.dma_start(jdg[:], jdiag[:].rearrange("p (g q) -> p g q", g=GT))
        nc.sync.dma_start(w1s[:], w1t[:])
        nc.sync.dma_start(w2s[:], w2t[:])
        sc_a, sc_b = cc[:, 0:1], cc[:, 1:2]
        sc_c1, sc_c2 = cc[:, 2:3], cc[:, 3:4]
        f0a, f1a = cc[:, 4:5], cc[:, 5:6]
        wBa, invba = cc[:, 6:7], cc[:, 7:8]

        id128h = pp.tile([P, P], F16)
        make_identity(nc, id128h[:])

        # fp8 top-k mask buffers (12 rows: mB x4 | mA*a x4 | mA*c1 x4, and the
        # rhs twin with b/c2 scales) so one DoubleRow psum chain yields
        # wB*SigmaB + wA*SigmaA after the wB post-scale (a*b+c1*c2 ~= wA/wB)
        ml = pp.tile([P, 12, N], FP8, name="ml")
        mr = pp.tile([P, 12, N], FP8, name="mr")
        # degrees at own rows
        d_own = pp.tile([P, 4], F32)
        d2_own = pp.tile([P, 4], F32)
        # slabT = Sigma_m[own 512 rows, all N cols] (fp16, post-RS)
        slabT = pp.tile([P, 4, N], F16)
        # layer activations
        u1 = pp.tile([P, 4, BF], F16)      # x@W1 at own rows (unscaled)
        u1d = pp.tile([P, 4, BF], F16)     # d * u1
        u1g = [pp.tile([P, 4, BF], F16, name=f"u1g{h}") for h in range(2)]
        u2d = pp.tile([P, 4, BF], F16)
        u2g = [pp.tile([P, 4, BF], F16, name=f"u2g{h}") for h in range(2)]

        # ================= mean features, fhat, sim prep =================
        with tc.tile_pool(name="prep", bufs=1) as sp:
            xTmy_s = sp.tile([C, B, 512], F32, tag="xTmy_s")
            nc.sync.dma_start(xTmy_s[:], xTmy[:])

            fmy = sp.tile([C, 512], F32)
            nc.vector.tensor_tensor(fmy[:], xTmy_s[:, 0], xTmy_s[:, 1], AL.add)
            nc.vector.tensor_tensor(fmy[:], fmy[:], xTmy_s[:, 2], AL.add)
            nc.vector.tensor_tensor(fmy[:], fmy[:], xTmy_s[:, 3], AL.add)

            fm = sp.tile([C, N], F32)
            fhat = sp.tile([C, N], F32, tag="fhat")
            with tc.tile_pool(name="xload", bufs=1) as xl:
                xTm_s = xl.tile([C, B, N], F32, tag="xTm_s")
                for b in range(B):
                    nc.sync.dma_start(xTm_s[:, b], xTm[:][:, b])
                nc.gpsimd.tensor_tensor(fm[:], xTm_s[:, 0], xTm_s[:, 1], AL.add)
                nc.gpsimd.tensor_tensor(fm[:], fm[:], xTm_s[:, 2], AL.add)
                nc.gpsimd.tensor_tensor(fm[:], fm[:], xTm_s[:, 3], AL.add)
                nc.gpsimd.tensor_scalar_mul(fm[:], fm[:], 0.25)

                fsq = xl.tile([C, N], F32, tag="fsq")
                nc.vector.tensor_tensor(fsq[:], fm[:], fm[:], AL.mult)
                onesC = sp.tile([C, 1], F32)
                nc.vector.memset(onesC[:], 1.0)
                nsq = xl.tile([1, N], F32, tag="nsq")
                with tc.tile_pool(name="psP", bufs=2, space="PSUM") as psP:
                    for ch in range(4):
                        ps = psP.tile([1, 512], F32, tag="ps1")
                        nc.tensor.matmul(ps[:], lhsT=onesC[:],
                                         rhs=fsq[:, ch * 512:(ch + 1) * 512],
                                         start=True, stop=True)
                        nc.scalar.activation(nsq[:, ch * 512:(ch + 1) * 512], ps[:], AF.Copy)
                nc.vector.tensor_scalar_max(nsq[:], nsq[:], 1e-24)
                nc.vector.reciprocal(nsq[:], nsq[:])
                nc.scalar.activation(nsq[:], nsq[:], AF.Sqrt)
                onesC1 = xl.tile([1, C], F32, tag="onesC1")
                nc.vector.memset(onesC1[:], 1.0)
                with tc.tile_pool(name="psB", bufs=1, space="PSUM") as psB:
                    psb = psB.tile([C, N], F32, tag="bc")
                    for ch in range(4):
                        nc.tensor.matmul(psb[:, ch * 512:(ch + 1) * 512],
                                         lhsT=onesC1[:],
                                         rhs=nsq[:, ch * 512:(ch + 1) * 512],
                                         start=True, stop=True)
                    nc.vector.tensor_tensor(fhat[:], fm[:], psb[:], AL.mult)

            # FM1 early: u1 = (x @ W1^T) at own rows (scale by d after RS)
            with tc.tile_pool(name="psU", bufs=2, space="PSUM") as psU:
                for t in range(4):
                    for b in range(B):
                        psy = psU.tile([P, F], F32, tag="psy")
                        nc.tensor.matmul(psy[:], lhsT=xTmy_s[:, b, t * P:(t + 1) * P],
                                         rhs=w1s[:], start=True, stop=True)
                        nc.scalar.activation(u1[:, t, b * F:(b + 1) * F], psy[:], AF.Copy)

            # ============ sim rows + top-k masks (own 4 tiles) ============
            with tc.tile_pool(name="topk", bufs=2) as tkp, \
                 tc.tile_pool(name="psS", bufs=2, space="PSUM") as psS:
                scr = sp.tile([P, 8], F32)
                inv8 = sp.tile([P, 8], F32)
                for t in range(4):
                    sim_sb = tkp.tile([P, N], F32, tag="simsb")
                    for ch in range(4):
                        ps = psS.tile([P, 512], F32, tag="ps512")
                        nc.tensor.matmul(ps[:], lhsT=fmy[:, t * P:(t + 1) * P],
                                         rhs=fhat[:, ch * 512:(ch + 1) * 512],
                                         start=True, stop=True)
                        nc.scalar.activation(sim_sb[:, ch * 512:(ch + 1) * 512], ps[:], AF.Copy)
                    work = tkp.tile([P, N], F32, tag="work")
                    src = sim_sb
                    for r in range(3):
                        nc.vector.max(out=scr[:], in_=src[:])
                        nc.vector.tensor_tensor(scr[:], scr[:], sm[:, r * 8:(r + 1) * 8], AL.mult)
                        nc.vector.tensor_scalar(inv8[:], sm[:, r * 8:(r + 1) * 8],
                                                -MINVAL, MINVAL, AL.mult, AL.add)
                        nc.vector.tensor_tensor(scr[:], scr[:], inv8[:], AL.add)
                        nc.vector.match_replace(out=work[:], in_to_replace=scr[:],
                                                in_values=src[:], imm_value=MINVAL)
                        src = work
                        if r == 0:
                            tA = tkp.tile([P, N], FP8, tag="tA")
                            nc.vector.tensor_tensor(tA[:], work[:], sim_sb[:],
                                                    AL.not_equal)
                    nc.vector.tensor_tensor(ml[:, t, :], work[:], sim_sb[:], AL.not_equal)
                    nc.gpsimd.tensor_copy(mr[:, t, :], ml[:, t, :])
                    nc.gpsimd.tensor_scalar(ml[:, 4 + t, :], tA[:], sc_a, None, AL.mult)
                    nc.gpsimd.tensor_scalar(mr[:, 4 + t, :], tA[:], sc_b, None, AL.mult)
                    nc.gpsimd.tensor_scalar(ml[:, 8 + t, :], tA[:], sc_c1, None, AL.mult)
                    nc.gpsimd.tensor_scalar(mr[:, 8 + t, :], tA[:], sc_c2, None, AL.mult)

        if DEBUG:
            nc.sync.dma_start(dbg_mk[:].rearrange("p (q t n) -> q p t n", q=2, t=4)[0],
                              ml[:, 4:8, :])
            nc.sync.dma_start(dbg_mk[:].rearrange("p (q t n) -> q p t n", q=2, t=4)[1],
                              ml[:, 0:4, :])

        # ============ partial Sigma + colsums -> rsinS -> RS ============
        with tc.tile_pool(name="sig", bufs=2) as sgp:
            ones8 = sgp.tile([P, 1], FP8, tag="ones8")
            nc.vector.memset(ones8[:], 1.0)
            # colsums: q=0 -> b*csA (mr rows 4..8), q=1 -> csB (mr rows 0..4)
            csf = sgp.tile([1, 2 * N], F16, tag="csf")
            with tc.tile_pool(name="psCS", bufs=2, space="PSUM") as psCS:
                for q, base in ((0, 4), (1, 0)):
                    for ch in range(4):
                        psc = psCS.tile([P, 512], F32, tag=f"cs{ch % 2}")
                        for t in range(4):
                            nc.tensor.matmul(psc[0:1, :], lhsT=ones8[:],
                                             rhs=mr[:, base + t,
                                                    ch * 512:(ch + 1) * 512],
                                             start=(t == 0), stop=(t == 3))
                        nc.scalar.activation(
                            csf[:, q * N + ch * 512:q * N + (ch + 1) * 512],
                            psc[0:1, :], AF.Copy)
            nc.sync.dma_start(csflat_d[:], csf[:])
            # scatter cs' into per-dest regions (dram->dram, contiguous rows)
            csv = csflat_d[:].rearrange("a (q n) -> (a q) n", q=2)
            for d in range(4):
                nc.sync.dma_start(
                    cs_in[d:d + 1, :].rearrange("a (q x) -> (a q) x", q=2),
                    csv[:, d * 512:(d + 1) * 512])

            # partial Sigma chunks via fp8 DoubleRow (6 pair-matmuls each)
            with tc.tile_pool(name="psSG", bufs=1, space="PSUM") as psSG:
                for mt in range(NT):
                    stg = sgp.tile([P, 4 * 512], F16, tag=f"stg{mt % 2}")
                    for ch in range(4):
                        psg = psSG.tile([P, 512], F32, tag=f"sg{ch}")
                        for i, j in enumerate((0, 2, 4, 1, 3, 5)):
                            nc.tensor.matmul(
                                psg[:], lhsT=ml[:, 2 * j:2 * j + 2, mt * P:(mt + 1) * P],
                                rhs=mr[:, 2 * j:2 * j + 2, ch * 512:(ch + 1) * 512],
                                start=(i == 0), stop=(i == 5),
                                perf_mode=mybir.MatmulPerfMode.DoubleRow)
                        nc.vector.tensor_scalar(stg[:, ch * 512:(ch + 1) * 512],
                                                psg[:], wBa, None, AL.mult)
                    nc.sync.dma_start(sig_in[mt // 4, mt % 4, :, :], stg[:])

            if not NO_COLLECTIVES:
                nc.gpsimd.collective_compute("ReduceScatter", AL.add, replica_groups=MODW,
                                             ins=[rsinS[:]], outs=[rsoutS[:]])

        # ============ post-RS: slabT, degrees, u1 variants ============
        with tc.tile_pool(name="post", bufs=1) as pq, \
             tc.tile_pool(name="psQ", bufs=2, space="PSUM") as psQ:
            nc.sync.dma_start(slabT[:], sig_out[:].rearrange("t p n -> p t n"))
            cst8 = pq.tile([8, P], F16, tag="cst8")
            nc.sync.dma_start(cst8[:],
                              cs_out[:].rearrange("a (k t p) -> (a k t) p", k=2, t=4, p=P))
            pst = psQ.tile([P, 8], F16, tag="pst")
            nc.tensor.transpose(pst[:], cst8[:], id128h[0:8, 0:8])
            cs_own = pq.tile([P, 8], F32, tag="cs_own")
            nc.scalar.activation(cs_own[:], pst[:], AF.Copy)
            dv = pq.tile([P, 4], F32, tag="dv")
            nc.vector.tensor_scalar(dv[:], cs_own[:, 0:4], invba, 1.0, AL.mult, AL.add)
            nc.vector.tensor_tensor(dv[:], dv[:], cs_own[:, 4:8], AL.add)
            nc.vector.reciprocal(d2_own[:], dv[:])
            nc.scalar.activation(d_own[:], d2_own[:], AF.Sqrt)

            if DEBUG:
                nc.sync.dma_start(dbg_slabT[:].rearrange("p (t n) -> p t n", t=4), slabT[:])
                nc.sync.dma_start(dbg_cs[:], cs_own[:])
                dcat = pq.tile([P, 8], F32, tag="dcat")
                nc.vector.tensor_copy(dcat[:, 0:4], d_own[:])
                nc.vector.tensor_copy(dcat[:, 4:8], d2_own[:])
                nc.sync.dma_start(dbg_d[:], dcat[:])

            # u1 variants: u1d = d*u1; u1g[h] = f_h * u1d
            for t in range(4):
                nc.vector.tensor_scalar(u1d[:, t], u1[:, t], d_own[:, t:t + 1],
                                        None, AL.mult)
            for h, fl in enumerate((f0a, f1a)):
                nc.gpsimd.tensor_scalar(u1g[h][:].rearrange("p t f -> p (t f)"),
                                        u1d[:].rearrange("p t f -> p (t f)"),
                                        fl, None, AL.mult)

        # ============ AGG1 -> RS1 ============
        with tc.tile_pool(name="agg1", bufs=2) as a1p, \
             tc.tile_pool(name="psA1", bufs=1, space="PSUM") as psA1:
            for blk in range(4):
                pss = [psA1.tile([P, BF], F32, tag=f"a{i}", name=f"ps1_{i}") for i in range(8)]
                for t in range(4):
                    for gg in range(4):
                        for half in range(2):
                            nc.tensor.matmul(
                                pss[gg * 2 + half],
                                lhsT=slabT[:, t, (blk * 4 + gg) * P:(blk * 4 + gg + 1) * P],
                                rhs=u1g[half][:, t], start=(t == 0), stop=False)
                for gg in range(4):
                    for half in range(2):
                        g = 16 * half + blk * 4 + gg
                        nc.tensor.matmul(pss[gg * 2 + half], lhsT=jdg[:, g, :],
                                         rhs=u1d[:, gg], start=False, stop=True)
                stages = [a1p.tile([P, 4 * BF], F16, tag=f"st{h}", name=f"st1_{h}") for h in range(2)]
                for gg in range(4):
                    for half in range(2):
                        dst = stages[half][:, gg * BF:(gg + 1) * BF]
                        if gg % 2 == 0:
                            nc.scalar.activation(dst, pss[gg * 2 + half], AF.Copy)
                        else:
                            nc.vector.tensor_copy(dst, pss[gg * 2 + half])
                for half in range(2):
                    nc.sync.dma_start(
                        rsin1[4 * half + blk].rearrange("t p f -> p t f"),
                        stages[half][:].rearrange("p (t f) -> p t f", t=4))
            if DEBUG:
                nc.sync.dma_start(dbg_u1d[:].rearrange("p (t f) -> p t f", t=4), u1d[:])
                nc.sync.dma_start(dbg_rsin1[:],
                                  rsin1[:].rearrange("d t p f -> (d t) (p f)"))
            if not NO_COLLECTIVES:
                nc.gpsimd.collective_compute("ReduceScatter", AL.add, replica_groups=ALLW,
                                             ins=[rsin1[:]], outs=[rsout1[:]])

        # ============ x1 = relu(d*y1); x1T; u2 = d2*(x1@W2^T) ============
        with tc.tile_pool(name="mid", bufs=1) as mp, \
             tc.tile_pool(name="psM", bufs=1, space="PSUM") as psM:
            y1 = mp.tile([P, 4, BF], F16, tag="y1")
            nc.sync.dma_start(y1[:], rsout1[:].rearrange("t p f -> p t f"))
            if DEBUG:
                nc.sync.dma_start(dbg_y1[:].rearrange("p (t f) -> p t f", t=4), y1[:])
            x1 = mp.tile([P, 4, BF], F16, tag="x1")
            for t in range(4):
                nc.scalar.activation(x1[:, t], y1[:, t], AF.Relu, scale=d_own[:, t:t + 1])
            x1T = mp.tile([P, 16, P], F16, tag="x1T")
            for t in range(4):
                for b in range(B):
                    pst2 = psM.tile([P, P], F16, tag=f"tr{(t * B + b) % 4}")
                    nc.tensor.transpose(pst2[:], x1[:, t, b * F:(b + 1) * F], id128h[:])
                    nc.scalar.activation(x1T[:, t * 4 + b, :], pst2[:], AF.Copy)
            for t in range(4):
                for b in range(B):
                    psy = psM.tile([P, F], F32, tag=f"fm{(t * B + b) % 4}")
                    nc.tensor.matmul(psy[:], lhsT=x1T[:, t * 4 + b, :], rhs=w2s[:],
                                     start=True, stop=True)
                    nc.scalar.activation(u2d[:, t, b * F:(b + 1) * F], psy[:], AF.Copy,
                                         scale=d_own[:, t:t + 1])
            for h, fl in enumerate((f0a, f1a)):
                nc.gpsimd.tensor_scalar(u2g[h][:].rearrange("p t f -> p (t f)"),
                                        u2d[:].rearrange("p t f -> p (t f)"),
                                        fl, None, AL.mult)

        # ============ AGG2 -> RS2 -> out ============
        with tc.tile_pool(name="agg2", bufs=2) as a2p, \
             tc.tile_pool(name="psA2", bufs=1, space="PSUM") as psA2:
            for blk in range(4):
                pss = [psA2.tile([P, BF], F32, tag=f"a{i}", name=f"ps2_{i}") for i in range(8)]
                for t in range(4):
                    for gg in range(4):
                        for half in range(2):
                            nc.tensor.matmul(
                                pss[gg * 2 + half],
                                lhsT=slabT[:, t, (blk * 4 + gg) * P:(blk * 4 + gg + 1) * P],
                                rhs=u2g[half][:, t], start=(t == 0), stop=False)
                for gg in range(4):
                    for half in range(2):
                        g = 16 * half + blk * 4 + gg
                        nc.tensor.matmul(pss[gg * 2 + half], lhsT=jdg[:, g, :],
                                         rhs=u2d[:, gg], start=False, stop=True)
                stages = [a2p.tile([P, 4 * BF], F16, tag=f"st{h}", name=f"st2_{h}") for h in range(2)]
                for gg in range(4):
                    for half in range(2):
                        dst = stages[half][:, gg * BF:(gg + 1) * BF]
                        if gg % 2 == 0:
                            nc.scalar.activation(dst, pss[gg * 2 + half], AF.Copy)
                        else:
                            nc.vector.tensor_copy(dst, pss[gg * 2 + half])
                for half in range(2):
                    nc.sync.dma_start(
                        rsin2[4 * half + blk].rearrange("t p f -> p t f"),
                        stages[half][:].rearrange("p (t f) -> p t f", t=4))
            if not NO_COLLECTIVES:
                nc.gpsimd.collective_compute("ReduceScatter", AL.add, replica_groups=ALLW,
                                             ins=[rsin2[:]], outs=[rsout2[:]])

        with tc.tile_pool(name="fin", bufs=1) as fp:
            y2 = fp.tile([P, 4, BF], F16, tag="y2")
            nc.sync.dma_start(y2[:], rsout2[:].rearrange("t p f -> p t f"))
            outsb = fp.tile([P, 4, B, F], F32, tag="outsb")
            for t in range(4):
                nc.scalar.activation(outsb[:, t].rearrange("p b f -> p (b f)"),
                                     y2[:, t], AF.Relu, scale=d_own[:, t:t + 1])
            for t in range(4):
                nc.sync.dma_start(
                    out_z[:, t * P:(t + 1) * P, :].rearrange("b p f -> p b f"),
                    outsb[:, t])

    nc.compile()
    return nc


def _make_inputs(feat_mod1, feat_mod2, W1, W2):
    f1 = np.ascontiguousarray(np.asarray(feat_mod1), np.float32)
    f2 = np.ascontiguousarray(np.asarray(feat_mod2), np.float32)
    xT1 = np.ascontiguousarray(f1.transpose(2, 0, 1))
    xT2 = np.ascontiguousarray(f2.transpose(2, 0, 1))
    w1t = np.ascontiguousarray(np.asarray(W1, np.float32).T)
    w2t = np.ascontiguousarray(np.asarray(W2, np.float32).T.astype(np.float16))

    KS = {0: (7, 19), 1: (5, 13)}  # k+1 per modality
    in_maps = []
    for c in range(8):
        m, s = c // 4, c % 4
        xTm = xT1 if m == 0 else xT2
        xTmy = np.ascontiguousarray(xTm[:, :, s * 512:(s + 1) * 512])
        kA, kB = KS[m]
        slotm = np.zeros((P, 24), np.float32)
        slotm[:, 0:kA] = 1.0
        slotm[:, 8:16] = 1.0
        rem = kB - kA - 8
        if rem > 0:
            slotm[:, 16:16 + rem] = 1.0
        f0 = 1.0 if m == 0 else 0.0
        a, bsc, c1, c2 = ((3.25, 2.25, 0.25, 0.21875) if m == 0 else
                          (2.25, 3.0, 0.125, 0.078125))
        ccv = np.zeros((P, 8), np.float32)
        ccv[:, 0] = a
        ccv[:, 1] = bsc
        ccv[:, 2] = c1
        ccv[:, 3] = c2
        ccv[:, 4] = f0
        ccv[:, 5] = 1.0 - f0
        ccv[:, 6] = 1.0 / (kB * kB)
        ccv[:, 7] = 1.0 / bsc
        jd = np.zeros((P, GT, P), np.float16)
        for g in range(GT):
            if s * 4 <= (g % 16) < (s + 1) * 4:
                jd[np.arange(P), g, np.arange(P)] = 0.25
        in_maps.append({
            "xTm": xTm, "xTmy": xTmy, "w1t": w1t, "w2t": w2t,
            "slotmask": slotm, "cconst": ccv,
            "jdiag": np.ascontiguousarray(jd.reshape(P, GT * P)),
        })
    return in_maps


def kernel(feat_mod1, feat_mod2, W1, W2):
    global _CACHED_NC, LAST_EXEC_TIME_NS, LAST_RESULTS
    if _CACHED_NC is None:
        _CACHED_NC = build_nc()
    in_maps = _make_inputs(feat_mod1, feat_mod2, W1, W2)
    res = run_bass_kernel_spmd(_CACHED_NC, in_maps, list(range(8)))
    LAST_RESULTS = res
    LAST_EXEC_TIME_NS = getattr(res, "exec_time_ns", None)
    outs = [res.results[c]["out_z"] for c in range(8)]
    out1 = np.concatenate(outs[0:4], axis=1)
    out2 = np.concatenate(outs[4:8], axis=1)
    return out1, out2
